# revision 1
# baseline (speedup 1.0000x reference)
"""GPT2 attention (B=2,S=2048,D=1024,H=16,hd=64, no causal mask) on 8 trn2 cores.

Sharding: core c handles batch b=c//4 and head-group g=c%4 (4 heads).
w_attn columns are split per head group (Q scaled by 1/sqrt(hd) on host);
w_proj rows split per head group; host sums the 4 partial c_proj outputs
per batch (the "all-reduce").

Per-core dataflow (matmuls in float32r, 1 cyc/row at N>=512; every tile a
matmul consumes is written as float32r by its producer so walrus' rounding
check passes):
  hid [2048,1024] --PE transpose--> hidT [1024,2048]
  qkvT[768,2048] = w_slice.T @ hidT   (feature-major Q^T,K^T,V^T, 2 heads/tile)
  V^T --PE transpose--> vaug [k,65] tiles (col 64 = ones for denominator)
  per (head, 512-wide q chunk):
    S^T[k,q] tiles = K^T_tile.T @ Q^T  -> DVE copy to SBUF block [128, 4096]
    one ACT exp per block (amortizes ACT fixed cost; no max-subtraction:
    scores are O(1) so exp is numerically safe)
    O_u^T[65,512] = sum_k vaug.T @ E   (row 64 = softmax denominator)
    obar_h = O_u^T[0:64] * broadcast(1/denom)  (ones-matmul broadcast + DVE mul)
  out[q,1024] = sum_h obar_h.T @ wp_h  (K=64 accumulation, 4 heads)
"""

import sys

import numpy as np

if "/opt/trn_rl_repo" not in sys.path:
    sys.path.insert(0, "/opt/trn_rl_repo")

S = 2048
D = 1024
P = 128
NH = 4  # heads per core
HD = 64
N_CORES = 8

_CACHE = {}


def _build_program():
    import concourse.mybir as mybir
    from concourse import bacc
    from concourse.masks import make_identity
    from concourse.tile import TileContext

    f32r = mybir.dt.float32r
    f32 = mybir.dt.float32
    AF = mybir.ActivationFunctionType
    ALU = mybir.AluOpType

    nc = bacc.Bacc(None, target_bir_lowering=False, debug=False)
    hid = nc.declare_dram_parameter("hid", [S, D], f32r, isOutput=False)
    wqkv = nc.declare_dram_parameter("wqkv", [D, 3 * NH * HD], f32r, isOutput=False)
    wp = nc.declare_dram_parameter("wp", [NH * HD, D], f32r, isOutput=False)
    out = nc.declare_dram_parameter("out", [S, D], f32, isOutput=True)

    with TileContext(nc) as tc:
        with tc.tile_pool(name="const", bufs=1) as constp:
            ident_f = constp.tile([P, P], f32)
            make_identity(nc, ident_f)
            ident = constp.tile([P, P], f32r)
            nc.vector.tensor_copy(ident[:], ident_f[:])
            ones_f = constp.tile([P, HD], f32)
            nc.gpsimd.memset(ones_f[:], 1.0)
            ones_t = constp.tile([P, HD], f32r)
            nc.vector.tensor_copy(ones_t[:], ones_f[:])

            qkvT = [constp.tile([P, S], f32r, name=f"qkvT{i}") for i in range(6)]
            vaug = constp.tile([P, NH * 16 * 65], f32r)

            # ---------------- Stage A: hidT + QKV ----------------
            with tc.tile_pool(name="hidT_pool", bufs=1) as hidTp, \
                 tc.tile_pool(name="stageA", bufs=3) as sA, \
                 tc.tile_pool(name="w_pool", bufs=1) as wpool, \
                 tc.tile_pool(name="tpsum", bufs=3, space="PSUM") as tpsum, \
                 tc.tile_pool(name="qpsum", bufs=3, space="PSUM") as qpsum:
                hidT = [hidTp.tile([P, S], f32r, name=f"hidT{i}") for i in range(8)]
                w_sb = [wpool.tile([P, 768], f32r, name=f"w{i}") for i in range(8)]
                for i in range(8):
                    nc.sync.dma_start(out=w_sb[i][:], in_=wqkv[i * P : (i + 1) * P, :])
                for st in range(16):
                    ht = sA.tile([P, D], f32r, tag="hidload")
                    nc.sync.dma_start(out=ht[:], in_=hid[st * P : (st + 1) * P, :])
                    for dt_ in range(8):
                        tp = tpsum.tile([P, P], f32r, tag="tp")
                        nc.tensor.transpose(
                            tp[:], ht[:, dt_ * P : (dt_ + 1) * P], ident[:]
                        )
                        nc.vector.tensor_copy(
                            hidT[dt_][:, st * P : (st + 1) * P], tp[:]
                        )
                for ct in range(6):
                    for qc in range(4):
                        ps = qpsum.tile([P, 512], f32, tag="qkvps")
                        for dt_ in range(8):
                            nc.tensor.matmul(
                                ps[:],
                                lhsT=w_sb[dt_][:, ct * P : (ct + 1) * P],
                                rhs=hidT[dt_][:, qc * 512 : (qc + 1) * 512],
                                start=(dt_ == 0),
                                stop=(dt_ == 7),
                            )
                        nc.vector.tensor_copy(
                            qkvT[ct][:, qc * 512 : (qc + 1) * 512], ps[:]
                        )
                # V seq-major (transpose V^T) into vaug; col 64 of each 65 = ones
                for h in range(NH):
                    par = HD * (h % 2)
                    vsrc = qkvT[4 + h // 2]
                    for kt in range(16):
                        vp = tpsum.tile([P, P], f32r, tag="tp")
                        nc.tensor.transpose(
                            vp[:, :HD],
                            vsrc[par : par + HD, kt * P : (kt + 1) * P],
                            ident[par : par + HD, par : par + HD],
                        )
                        base = (h * 16 + kt) * 65
                        nc.vector.tensor_copy(vaug[:, base : base + HD], vp[:, :HD])
                        nc.vector.tensor_copy(
                            vaug[:, base + HD : base + 65], ones_f[:, 0:1]
                        )

            # ---------------- Stages B+C ----------------
            with tc.tile_pool(name="persistBC", bufs=1) as perBC:
                obar = [perBC.tile([HD, S], f32r, name=f"obar{i}") for i in range(NH)]
                wp_sb = [perBC.tile([HD, D], f32r, name=f"wp{i}") for i in range(NH)]
                for h in range(NH):
                    nc.sync.dma_start(
                        out=wp_sb[h][:], in_=wp[h * HD : (h + 1) * HD, :]
                    )

                with tc.tile_pool(name="sblk", bufs=3) as sblk, \
                     tc.tile_pool(name="npool", bufs=3) as npool, \
                     tc.tile_pool(name="spsum", bufs=2, space="PSUM") as spsum, \
                     tc.tile_pool(name="opsum", bufs=1, space="PSUM") as opsum, \
                     tc.tile_pool(name="rpsum", bufs=1, space="PSUM") as rpsum:
                    for h in range(NH):
                        par = HD * (h % 2)
                        qT = qkvT[0 + h // 2]
                        kT = qkvT[2 + h // 2]
                        for qc in range(2):
                            q0 = qc * 1024
                            op = opsum.tile([65, 1024], f32, tag="op")
                            for kt in range(16):
                                sp = spsum.tile([P, 1024], f32, tag="sp")
                                for u in range(2):
                                    nc.tensor.matmul(
                                        sp[:, u * 512 : (u + 1) * 512],
                                        lhsT=kT[par : par + HD, kt * P : (kt + 1) * P],
                                        rhs=qT[par : par + HD, q0 + u * 512 : q0 + (u + 1) * 512],
                                        start=True,
                                        stop=True,
                                    )
                                eb = sblk.tile([P, 1024], f32r, tag="sb")
                                nc.scalar.activation(eb[:], sp[:], AF.Exp)
                                base = (h * 16 + kt) * 65
                                for u in range(2):
                                    nc.tensor.matmul(
                                        op[:, u * 512 : (u + 1) * 512],
                                        lhsT=vaug[:, base : base + 65],
                                        rhs=eb[:, u * 512 : (u + 1) * 512],
                                        start=(kt == 0),
                                        stop=(kt == 15),
                                    )
                            rec = npool.tile([P, 1024], f32r, tag="rec")
                            with nc.allow_low_precision(
                                reason="f32r recip of softmax denom"
                            ):
                                nc.vector.reciprocal(rec[64:65, :], op[64:65, :])
                            rb = rpsum.tile([HD, 1024], f32, tag="rb")
                            for u in range(2):
                                nc.tensor.matmul(
                                    rb[:, u * 512 : (u + 1) * 512],
                                    lhsT=ones_t[64:65, :],
                                    rhs=rec[64:65, u * 512 : (u + 1) * 512],
                                    start=True, stop=True,
                                )
                            ou_sb = npool.tile([HD, 1024], f32r, tag="ou")
                            nc.vector.tensor_copy(ou_sb[:], op[0:HD, :])
                            rb_sb = npool.tile([HD, 1024], f32r, tag="rbs")
                            nc.vector.tensor_copy(rb_sb[:], rb[:])
                            with nc.allow_low_precision(
                                reason="softmax normalize in f32r"
                            ):
                                nc.vector.tensor_tensor(
                                    out=obar[h][:, q0 : q0 + 1024],
                                    in0=ou_sb[:],
                                    in1=rb_sb[:],
                                    op=ALU.mult,
                                )

                # ---------------- Stage C: projection ----------------
                with tc.tile_pool(name="outp", bufs=4) as outp, \
                     tc.tile_pool(name="ppsum", bufs=4, space="PSUM") as ppsum:
                    for qt in range(16):
                        ot = outp.tile([P, D], f32, tag="ot")
                        for ec in range(2):
                            pp = ppsum.tile([P, 512], f32, tag="pp")
                            for h in range(NH):
                                nc.tensor.matmul(
                                    pp[:],
                                    lhsT=obar[h][:, qt * P : (qt + 1) * P],
                                    rhs=wp_sb[h][:, ec * 512 : (ec + 1) * 512],
                                    start=(h == 0),
                                    stop=(h == NH - 1),
                                )
                            nc.vector.tensor_copy(
                                ot[:, ec * 512 : (ec + 1) * 512], pp[:]
                            )
                        nc.sync.dma_start(
                            out=out[qt * P : (qt + 1) * P, :], in_=ot[:]
                        )

    nc.compile()
    return nc


def _get_nc():
    if "nc" not in _CACHE:
        _CACHE["nc"] = _build_program()
    return _CACHE["nc"]


def _shard_inputs(hidden_states, w_attn, w_proj):
    scale = 1.0 / np.sqrt(np.float32(HD))
    in_maps = []
    for c in range(N_CORES):
        b, g = divmod(c, 4)
        cs = slice(g * NH * HD, (g + 1) * NH * HD)
        wq = w_attn[:, 0:D][:, cs] * scale
        wk = w_attn[:, D : 2 * D][:, cs]
        wv = w_attn[:, 2 * D : 3 * D][:, cs]
        in_maps.append(
            {
                "hid": np.ascontiguousarray(hidden_states[b], dtype=np.float32),
                "wqkv": np.ascontiguousarray(
                    np.concatenate([wq, wk, wv], axis=1), dtype=np.float32
                ),
                "wp": np.ascontiguousarray(w_proj[cs, :], dtype=np.float32),
            }
        )
    return in_maps


def run(hidden_states, w_attn, w_proj, trace=False):
    from concourse.bass_utils import run_bass_kernel_spmd

    nc = _get_nc()
    in_maps = _shard_inputs(hidden_states, w_attn, w_proj)
    res = run_bass_kernel_spmd(nc, in_maps, list(range(N_CORES)), trace=trace)
    parts = [res.results[c]["out"] for c in range(N_CORES)]
    out = np.stack(
        [
            parts[0] + parts[1] + parts[2] + parts[3],
            parts[4] + parts[5] + parts[6] + parts[7],
        ]
    ).astype(np.float32)
    return out, res


def kernel(hidden_states, w_attn, w_proj):
    out, _ = run(
        np.asarray(hidden_states), np.asarray(w_attn), np.asarray(w_proj)
    )
    return out



# revision 2
# speedup vs baseline: 1.0635x; 1.0635x over previous
"""GPT2 attention (B=2,S=2048,D=1024,H=16,hd=64, no causal mask) on 8 trn2 cores.

Sharding: core c handles batch b=c//4 and head-group g=c%4 (4 heads).
All device data is bf16 (halves transfer + SBUF footprint vs fp32);
matmul accumulation stays fp32 in PSUM. Host pre-transposes hidden
states (hidT upload) so no PE transposes are needed, pre-scales w_q by
1/sqrt(hd), and sums the 4 partial c_proj outputs per batch.

Per-core dataflow:
  hidT [1024,2048] bf16 (uploaded transposed)
  Q^T,K^T feature-major [128,2048] tiles (2 heads/tile) = w_chunk.T @ hidT
  V seq-major via matmul (lhsT=hidT chunk): vaug [k,65] blocks (col 64=1 for
    the softmax denominator, via one memset of the whole vaug to 1.0)
  per (head-pair, 1024-wide q chunk, 16 k tiles):
    S^T[k,q] pair computed with row-tiled concurrent matmuls (head A rows
    0-63, head B rows 64-127) into one [128,2048] fp32 PSUM tile
    one ACT exp per [128,2048] block -> bf16 SBUF (amortizes ACT overhead)
    O^T[65,1024] per head = sum_k vaug.T @ E  (row 64 = denominator)
    normalize: DVE fast reciprocal of denoms, GPSIMD partition_broadcast,
    DVE multiply -> obar[h] [64,2048] bf16
  out[q,1024] = sum_h obar_h.T @ wp_h  (K=64 accumulation, 4 heads)
"""

import sys

import numpy as np

if "/opt/trn_rl_repo" not in sys.path:
    sys.path.insert(0, "/opt/trn_rl_repo")

S = 2048
D = 1024
P = 128
NH = 4  # heads per core
HD = 64
N_CORES = 8

_CACHE = {}


def _build_program():
    import concourse.mybir as mybir
    from concourse import bacc
    from concourse.tile import TileContext

    bf16 = mybir.dt.bfloat16
    f32 = mybir.dt.float32
    AF = mybir.ActivationFunctionType
    ALU = mybir.AluOpType

    nc = bacc.Bacc(None, target_bir_lowering=False, debug=False)
    hidT = nc.declare_dram_parameter("hidT", [D, S], bf16, isOutput=False)
    wqkv = nc.declare_dram_parameter("wqkv", [D, 3 * NH * HD], bf16, isOutput=False)
    wp = nc.declare_dram_parameter("wp", [NH * HD, D], bf16, isOutput=False)
    out = nc.declare_dram_parameter("out", [S, D], bf16, isOutput=True)

    with TileContext(nc) as tc:
        with tc.tile_pool(name="persist", bufs=1) as per:
            # V (seq-major) + ones column per (head, ktile): 65-wide blocks
            vaug = per.tile([P, NH * 16 * 65], bf16)
            nc.gpsimd.memset(vaug[:], 1.0)
            # Q^T/K^T feature-major, 2 heads per tile: 0=Q01 1=Q23 2=K01 3=K23
            qkT = [per.tile([P, S], bf16, name=f"qkT{i}") for i in range(4)]
            # normalized attention output^T per head
            obar = [per.tile([HD, S], bf16, name=f"obar{i}") for i in range(NH)]
            wp_sb = [per.tile([HD, D], bf16, name=f"wp{i}") for i in range(NH)]
            for h in range(NH):
                nc.sync.dma_start(out=wp_sb[h][:], in_=wp[h * HD : (h + 1) * HD, :])

            # ---------------- Stage A: QK feature-major + V seq-major --------
            with tc.tile_pool(name="hidT_pool", bufs=1) as hp, \
                 tc.tile_pool(name="w_pool", bufs=1) as wpool, \
                 tc.tile_pool(name="qkpsum", bufs=3, space="PSUM") as qkp, \
                 tc.tile_pool(name="vpsum", bufs=3, space="PSUM") as vp_:
                hT = [hp.tile([P, S], bf16, name=f"hT{i}") for i in range(8)]
                w_sb = [wpool.tile([P, 768], bf16, name=f"w{i}") for i in range(8)]
                for i in range(8):
                    nc.sync.dma_start(out=w_sb[i][:], in_=wqkv[i * P : (i + 1) * P, :])
                    nc.sync.dma_start(out=hT[i][:], in_=hidT[i * P : (i + 1) * P, :])
                for ct in range(4):
                    for qc in range(4):
                        ps = qkp.tile([P, 512], f32, tag="qk")
                        for dt_ in range(8):
                            nc.tensor.matmul(
                                ps[:],
                                lhsT=w_sb[dt_][:, ct * P : (ct + 1) * P],
                                rhs=hT[dt_][:, qc * 512 : (qc + 1) * 512],
                                start=(dt_ == 0),
                                stop=(dt_ == 7),
                            )
                        nc.vector.tensor_copy(
                            qkT[ct][:, qc * 512 : (qc + 1) * 512], ps[:]
                        )
                for kt in range(16):
                    vps = vp_.tile([P, NH * HD], f32, tag="v")
                    for dt_ in range(8):
                        nc.tensor.matmul(
                            vps[:],
                            lhsT=hT[dt_][:, kt * P : (kt + 1) * P],
                            rhs=w_sb[dt_][:, 512:768],
                            start=(dt_ == 0),
                            stop=(dt_ == 7),
                        )
                    for h in range(NH):
                        base = (h * 16 + kt) * 65
                        nc.vector.tensor_copy(
                            vaug[:, base : base + HD], vps[:, h * HD : (h + 1) * HD]
                        )

            # ---------------- Stage B: attention ----------------
            with tc.tile_pool(name="ebpool", bufs=2) as ebp, \
                 tc.tile_pool(name="nrm", bufs=2) as nrm, \
                 tc.tile_pool(name="sps", bufs=1, space="PSUM") as sps, \
                 tc.tile_pool(name="opA", bufs=1, space="PSUM") as opAp, \
                 tc.tile_pool(name="opB", bufs=1, space="PSUM") as opBp:
                for qc in range(2):
                    q0 = qc * 1024
                    for p in range(2):
                        hA, hB = 2 * p, 2 * p + 1
                        qT, kT = qkT[p], qkT[2 + p]
                        opA = opAp.tile([65, 1024], f32, tag="opA")
                        opB = opBp.tile([65, 1024], f32, tag="opB")
                        for kt in range(16):
                            ks = slice(kt * P, (kt + 1) * P)
                            sp = sps.tile([P, 2048], f32, tag="sp")
                            # row-tiled concurrent pair: A rows 0-63, B rows 64-127
                            for u in range(2):
                                qs = slice(q0 + u * 512, q0 + (u + 1) * 512)
                                nc.tensor.matmul(
                                    sp[:, u * 512 : (u + 1) * 512],
                                    lhsT=kT[0:HD, ks],
                                    rhs=qT[0:HD, qs],
                                    start=True, stop=True,
                                )
                                nc.tensor.matmul(
                                    sp[:, 1024 + u * 512 : 1024 + (u + 1) * 512],
                                    lhsT=kT[HD:P, ks],
                                    rhs=qT[HD:P, qs],
                                    start=True, stop=True,
                                )
                            eb = ebp.tile([P, 2048], bf16, tag="eb")
                            nc.scalar.activation(eb[:], sp[:], AF.Exp)
                            bA = (hA * 16 + kt) * 65
                            bB = (hB * 16 + kt) * 65
                            for u in range(2):
                                us = slice(u * 512, (u + 1) * 512)
                                nc.tensor.matmul(
                                    opA[:, us],
                                    lhsT=vaug[:, bA : bA + 65],
                                    rhs=eb[:, u * 512 : (u + 1) * 512],
                                    start=(kt == 0), stop=(kt == 15),
                                )
                                nc.tensor.matmul(
                                    opB[:, us],
                                    lhsT=vaug[:, bB : bB + 65],
                                    rhs=eb[:, 1024 + u * 512 : 1024 + (u + 1) * 512],
                                    start=(kt == 0), stop=(kt == 15),
                                )
                        # normalize both heads of the pair
                        den = nrm.tile([1, 2048], f32, tag="den")
                        nc.vector.tensor_copy(den[0:1, 0:1024], opA[64:65, :])
                        nc.vector.tensor_copy(den[0:1, 1024:2048], opB[64:65, :])
                        rcp = nrm.tile([1, 2048], f32, tag="rcp")
                        nc.vector.reciprocal_approx_fast(rcp[:], den[:])
                        rbc = nrm.tile([HD, 2048], f32, tag="rbc")
                        nc.gpsimd.partition_broadcast(rbc[:], rcp[0:1, :])
                        with nc.allow_low_precision(reason="softmax normalize bf16"):
                            nc.vector.tensor_tensor(
                                out=obar[hA][:, q0 : q0 + 1024],
                                in0=opA[0:HD, :],
                                in1=rbc[:, 0:1024],
                                op=ALU.mult,
                            )
                            nc.vector.tensor_tensor(
                                out=obar[hB][:, q0 : q0 + 1024],
                                in0=opB[0:HD, :],
                                in1=rbc[:, 1024:2048],
                                op=ALU.mult,
                            )

            # ---------------- Stage C: projection ----------------
            with tc.tile_pool(name="outp", bufs=4) as outp, \
                 tc.tile_pool(name="ppsum", bufs=4, space="PSUM") as ppsum:
                for qt in range(16):
                    ot = outp.tile([P, D], bf16, tag="ot")
                    for ec in range(2):
                        pp = ppsum.tile([P, 512], f32, tag="pp")
                        for h in range(NH):
                            nc.tensor.matmul(
                                pp[:],
                                lhsT=obar[h][:, qt * P : (qt + 1) * P],
                                rhs=wp_sb[h][:, ec * 512 : (ec + 1) * 512],
                                start=(h == 0),
                                stop=(h == NH - 1),
                            )
                        nc.vector.tensor_copy(
                            ot[:, ec * 512 : (ec + 1) * 512], pp[:]
                        )
                    nc.sync.dma_start(out=out[qt * P : (qt + 1) * P, :], in_=ot[:])

    nc.compile()
    return nc


def _get_nc():
    if "nc" not in _CACHE:
        _CACHE["nc"] = _build_program()
    return _CACHE["nc"]


def _shard_inputs(hidden_states, w_attn, w_proj):
    import ml_dtypes

    bf16 = ml_dtypes.bfloat16
    scale = 1.0 / np.sqrt(np.float32(HD))
    hidT_b = [
        np.ascontiguousarray(hidden_states[b].T).astype(bf16) for b in range(2)
    ]
    in_maps = []
    for c in range(N_CORES):
        b, g = divmod(c, 4)
        cs = slice(g * NH * HD, (g + 1) * NH * HD)
        wq = w_attn[:, 0:D][:, cs] * scale
        wk = w_attn[:, D : 2 * D][:, cs]
        wv = w_attn[:, 2 * D : 3 * D][:, cs]
        in_maps.append(
            {
                "hidT": hidT_b[b],
                "wqkv": np.ascontiguousarray(
                    np.concatenate([wq, wk, wv], axis=1)
                ).astype(bf16),
                "wp": np.ascontiguousarray(w_proj[cs, :]).astype(bf16),
            }
        )
    return in_maps


def run(hidden_states, w_attn, w_proj, trace=False):
    from concourse.bass_utils import run_bass_kernel_spmd

    nc = _get_nc()
    in_maps = _shard_inputs(hidden_states, w_attn, w_proj)
    res = run_bass_kernel_spmd(nc, in_maps, list(range(N_CORES)), trace=trace)
    parts = [res.results[c]["out"].astype(np.float32) for c in range(N_CORES)]
    out = np.stack(
        [
            parts[0] + parts[1] + parts[2] + parts[3],
            parts[4] + parts[5] + parts[6] + parts[7],
        ]
    ).astype(np.float32)
    return out, res


def kernel(hidden_states, w_attn, w_proj):
    out, _ = run(
        np.asarray(hidden_states), np.asarray(w_attn), np.asarray(w_proj)
    )
    return out


# revision 4
# speedup vs baseline: 1.8362x; 1.7266x over previous
"""GPT2 attention (B=2,S=2048,D=1024,H=16,hd=64, no causal mask) on 8 trn2 cores.

Sharding: core c handles batch b=c//4 and head-group g=c%4 (4 heads).
All device data is bf16 (halves transfer + SBUF footprint vs fp32);
matmul accumulation stays fp32 in PSUM. Host pre-transposes hidden
states (hidT upload) so no PE transposes are needed, pre-scales w_q by
1/sqrt(hd), and sums the 4 partial c_proj outputs per batch.

The emission is software-pipelined to keep the tensor engine busy
continuously (HAM stays at K=8/8): stage-A leftovers (Q/K chunks) and
stage-C projection tiles are interleaved as "filler" units inside the
attention loop, whose pace is set by the ACT engine's exp throughput.

Per-core dataflow:
  hidT [1024,2048] bf16 (uploaded transposed)
  Q^T,K^T feature-major [128,2048] tiles (2 heads/tile) = w_chunk.T @ hidT
  V seq-major via matmul (lhsT=hidT chunk): vaug [k,65] blocks (col 64=1 for
    the softmax denominator, via one memset of the whole vaug to 1.0)
  per (head-pair p, 512-wide q chunk, 16 k tiles):
    S^T[k,q] for both heads with row-tiled concurrent matmuls (head A rows
    0-63, head B rows 64-127) into one [128,1024] fp32 PSUM tile
    one ACT exp per [128,1024] block -> bf16 SBUF
    O^T[65,512] per head += vaug.T @ E  (row 64 = softmax denominator)
    normalize: one DVE copy to SBUF staging, fast reciprocal of denoms,
    GPSIMD partition_broadcast, DVE multiply -> obar[h] [64,2048] bf16
  out[q,1024] = sum_h obar_h.T @ wp_h  (K=64 accumulation, 4 heads)
"""

import sys

import numpy as np

if "/opt/trn_rl_repo" not in sys.path:
    sys.path.insert(0, "/opt/trn_rl_repo")

S = 2048
D = 1024
P = 128
NH = 4  # heads per core
HD = 64
N_CORES = 8

_CACHE = {}


def _build_program():
    import concourse.mybir as mybir
    from concourse import bacc
    from concourse.tile import TileContext

    bf16 = mybir.dt.bfloat16
    f32 = mybir.dt.float32
    AF = mybir.ActivationFunctionType
    ALU = mybir.AluOpType

    nc = bacc.Bacc(None, target_bir_lowering=False, debug=False)
    hidT = nc.declare_dram_parameter("hidT", [D, S], bf16, isOutput=False)
    wqkv = nc.declare_dram_parameter("wqkv", [D, 3 * NH * HD], bf16, isOutput=False)
    wp = nc.declare_dram_parameter("wp", [NH * HD, D], bf16, isOutput=False)
    out = nc.declare_dram_parameter("out", [S, D], bf16, isOutput=True)

    with TileContext(nc) as tc:
        with tc.tile_pool(name="persist", bufs=1) as per, \
             tc.tile_pool(name="ebp", bufs=2) as ebp, \
             tc.tile_pool(name="stgp", bufs=2) as stgp, \
             tc.tile_pool(name="nrm", bufs=2) as nrm, \
             tc.tile_pool(name="outp", bufs=4) as outp, \
             tc.tile_pool(name="psum", bufs=1, space="PSUM") as psum:
            # V (seq-major) + ones column per (head, ktile): 65-wide blocks
            vaug = per.tile([P, NH * 16 * 65], bf16)
            nc.gpsimd.memset(vaug[:], 1.0)
            # Q^T/K^T feature-major, 2 heads per tile: 0=Q01 1=Q23 2=K01 3=K23
            qkT = [per.tile([P, S], bf16, name=f"qkT{i}") for i in range(4)]
            # normalized attention output^T per head
            obar = [per.tile([HD, S], bf16, name=f"obar{i}") for i in range(NH)]
            wp_sb = [per.tile([HD, D], bf16, name=f"wp{i}") for i in range(NH)]
            hT = [per.tile([P, S], bf16, name=f"hT{i}") for i in range(8)]
            w_sb = [per.tile([P, 768], bf16, name=f"w{i}") for i in range(8)]
            for h in range(NH):
                nc.sync.dma_start(out=wp_sb[h][:], in_=wp[h * HD : (h + 1) * HD, :])
            for i in range(8):
                nc.sync.dma_start(out=w_sb[i][:], in_=wqkv[i * P : (i + 1) * P, :])
                nc.sync.dma_start(out=hT[i][:], in_=hidT[i * P : (i + 1) * P, :])

            def qk_unit(ct, qc):
                # qkT[ct][:, qc*512:+512] = w[:, ct]^T @ hidT[:, qc chunk]
                ps = psum.tile([P, 512], f32, tag="aux", bufs=2)
                for dt_ in range(8):
                    nc.tensor.matmul(
                        ps[:],
                        lhsT=w_sb[dt_][:, ct * P : (ct + 1) * P],
                        rhs=hT[dt_][:, qc * 512 : (qc + 1) * 512],
                        start=(dt_ == 0),
                        stop=(dt_ == 7),
                    )
                nc.vector.tensor_copy(qkT[ct][:, qc * 512 : (qc + 1) * 512], ps[:])

            def v_unit(kt):
                # V rows kt*128.. for all 4 heads, scattered into vaug
                ps = psum.tile([P, 512], f32, tag="aux", bufs=2)
                for dt_ in range(8):
                    nc.tensor.matmul(
                        ps[:, 0 : NH * HD],
                        lhsT=hT[dt_][:, kt * P : (kt + 1) * P],
                        rhs=w_sb[dt_][:, 512:768],
                        start=(dt_ == 0),
                        stop=(dt_ == 7),
                    )
                for h in range(NH):
                    base = (h * 16 + kt) * 65
                    nc.vector.tensor_copy(
                        vaug[:, base : base + HD], ps[:, h * HD : (h + 1) * HD]
                    )

            def c_unit(qt):
                # out rows qt*128.. = sum_h obar_h^T @ wp_h
                ot = outp.tile([P, D], bf16, tag="ot")
                for ec in range(2):
                    pp = psum.tile([P, 512], f32, tag="aux", bufs=2)
                    for h in range(NH):
                        nc.tensor.matmul(
                            pp[:],
                            lhsT=obar[h][:, qt * P : (qt + 1) * P],
                            rhs=wp_sb[h][:, ec * 512 : (ec + 1) * 512],
                            start=(h == 0),
                            stop=(h == NH - 1),
                        )
                    nc.vector.tensor_copy(ot[:, ec * 512 : (ec + 1) * 512], pp[:])
                nc.sync.dma_start(out=out[qt * P : (qt + 1) * P, :], in_=ot[:])

            def b_block(p, qc, fillers):
                hA, hB = 2 * p, 2 * p + 1
                qT, kT = qkT[p], qkT[2 + p]
                q0 = qc * 512
                qs = slice(q0, q0 + 512)
                opq = psum.tile([65, 1024], f32, tag="op", bufs=1)
                for kt in range(16):
                    ks = slice(kt * P, (kt + 1) * P)
                    sp = psum.tile([P, 1024], f32, tag="sp", bufs=2)
                    # row-tiled concurrent pair: A rows 0-63, B rows 64-127
                    nc.tensor.matmul(
                        sp[:, 0:512], lhsT=kT[0:HD, ks], rhs=qT[0:HD, qs],
                        start=True, stop=True,
                    )
                    nc.tensor.matmul(
                        sp[:, 512:1024], lhsT=kT[HD:P, ks], rhs=qT[HD:P, qs],
                        start=True, stop=True,
                    )
                    eb = ebp.tile([P, 1024], bf16, tag="eb")
                    nc.scalar.activation(eb[:], sp[:], AF.Exp)
                    bA = (hA * 16 + kt) * 65
                    bB = (hB * 16 + kt) * 65
                    nc.tensor.matmul(
                        opq[:, 0:512], lhsT=vaug[:, bA : bA + 65],
                        rhs=eb[:, 0:512], start=(kt == 0), stop=(kt == 15),
                    )
                    nc.tensor.matmul(
                        opq[:, 512:1024], lhsT=vaug[:, bB : bB + 65],
                        rhs=eb[:, 512:1024], start=(kt == 0), stop=(kt == 15),
                    )
                    if kt % 4 == 3 and fillers:
                        fillers.pop(0)()
                # normalize both heads: evacuate PSUM fast, then SBUF-side math
                stg = stgp.tile([65, 1024], f32, tag="stg")
                nc.vector.tensor_copy(stg[:], opq[:])
                rcp = nrm.tile([1, 1024], f32, tag="rcp")
                nc.vector.reciprocal_approx_fast(rcp[:], stg[64:65, :])
                rbc = nrm.tile([HD, 1024], f32, tag="rbc")
                nc.gpsimd.partition_broadcast(rbc[:], rcp[0:1, :])
                with nc.allow_low_precision(reason="softmax normalize bf16"):
                    nc.vector.tensor_tensor(
                        out=obar[hA][:, qs], in0=stg[0:HD, 0:512],
                        in1=rbc[:, 0:512], op=ALU.mult,
                    )
                    nc.vector.tensor_tensor(
                        out=obar[hB][:, qs], in0=stg[0:HD, 512:1024],
                        in1=rbc[:, 512:1024], op=ALU.mult,
                    )

            # ---- stage A head: K01 (all chunks), Q01 chunk 0, V (all) ----
            for qc in range(4):
                qk_unit(2, qc)
            qk_unit(0, 0)
            for kt in range(16):
                v_unit(kt)

            # ---- remaining A-stage work (BISECT: no filler interleave) ----
            for ct, qc in [(0, 1), (0, 2), (3, 0), (0, 3), (3, 1), (3, 2),
                           (3, 3), (1, 0), (1, 1), (1, 2), (1, 3)]:
                qk_unit(ct, qc)

            for qc in range(4):
                b_block(0, qc, [])
            for qc in range(4):
                b_block(1, qc, [])
            for qt in range(16):
                c_unit(qt)

    nc.compile()
    return nc


def _get_nc():
    if "nc" not in _CACHE:
        _CACHE["nc"] = _build_program()
    return _CACHE["nc"]


def _shard_inputs(hidden_states, w_attn, w_proj):
    import ml_dtypes

    bf16 = ml_dtypes.bfloat16
    scale = 1.0 / np.sqrt(np.float32(HD))
    hidT_b = [
        np.ascontiguousarray(hidden_states[b].T).astype(bf16) for b in range(2)
    ]
    in_maps = []
    for c in range(N_CORES):
        b, g = divmod(c, 4)
        cs = slice(g * NH * HD, (g + 1) * NH * HD)
        wq = w_attn[:, 0:D][:, cs] * scale
        wk = w_attn[:, D : 2 * D][:, cs]
        wv = w_attn[:, 2 * D : 3 * D][:, cs]
        in_maps.append(
            {
                "hidT": hidT_b[b],
                "wqkv": np.ascontiguousarray(
                    np.concatenate([wq, wk, wv], axis=1)
                ).astype(bf16),
                "wp": np.ascontiguousarray(w_proj[cs, :]).astype(bf16),
            }
        )
    return in_maps


def run(hidden_states, w_attn, w_proj, trace=False):
    from concourse.bass_utils import run_bass_kernel_spmd

    nc = _get_nc()
    in_maps = _shard_inputs(hidden_states, w_attn, w_proj)
    res = run_bass_kernel_spmd(nc, in_maps, list(range(N_CORES)), trace=trace)
    parts = [res.results[c]["out"].astype(np.float32) for c in range(N_CORES)]
    out = np.stack(
        [
            parts[0] + parts[1] + parts[2] + parts[3],
            parts[4] + parts[5] + parts[6] + parts[7],
        ]
    ).astype(np.float32)
    return out, res


def kernel(hidden_states, w_attn, w_proj):
    out, _ = run(
        np.asarray(hidden_states), np.asarray(w_attn), np.asarray(w_proj)
    )
    return out
